# revision 39
# baseline (speedup 1.0000x reference)
"""Multi-head attention (B=2, N=2048, D=1024, H=16, dh=64) on 8 TRN2 cores.

Sharding: tensor-parallel over heads -- 2 heads per core. Each core computes
its heads' Q/K/V projections, attention, and a partial output projection
(rows of Wo for its heads); the host sums the 8 partial outputs (bf16).

Per-core layout strategy (v2 -- engine-balanced, HAM-warm):
  - Projections contract D with W stationary, producing Q^T/K^T/V^T
    ([local_dim, tok]); bias-adds run on the Scalar engine (idle then).
  - Scores: per k-tile, the two heads' [128,512] score matmuls are
    row-tiled (K=64 at array rows 0-63 / 64-127) into one [128,1024]
    two-bank PSUM tile; ONE exp activation covers both heads.
  - V is transposed via PE and packed into one vaug tile per batch:
    [V0 | 1 | V1 | 0 | 1] so head0's ctx matmul (M=65) puts Z0 at PSUM
    partition 64 and head1's (M=66) puts Z1 at partition 65 -- the two
    denominator rows land on distinct partitions and need only ONE
    reciprocal per q-slice.
  - ctx rows are evacuated to SBUF immediately (frees PSUM), normalized
    by a DRAM-roundtrip broadcast of 1/Z, and fed to the Wo matmul.
  - Partial outputs are written bf16 (halves writeback DMA).
"""

import numpy as np
import ml_dtypes
from contextlib import ExitStack

import concourse.bass as bass
import concourse.tile as tile
from concourse.tile_rust import add_dep_helper
from concourse import bacc, mybir
from concourse.bass import ts, ds
from concourse.bass_utils import run_bass_kernel_spmd
from concourse.masks import make_identity

BF16 = mybir.dt.bfloat16
F16 = mybir.dt.float16
F32 = mybir.dt.float32

B = 2
N = 2048          # tokens per batch
D = 1024          # model dim
NCORES = 8
DLOC = 128        # local dims per core (2 heads x 64)
DH = 64
QS = 512          # q slice
NQS = N // QS     # 4 per batch
NKT = N // 128    # 16 k-tiles of 128
NDCH = D // 128   # 8 d-chunks
VW = 131          # vaug width: V0(64) | zero | ones | V1(64) | ones


def _build_program():
    nc = bacc.Bacc("TRN2", target_bir_lowering=False, debug=False)

    xT = {}
    w = {}
    bias = {}
    for t in ("q", "k", "v"):
        xT[t] = nc.dram_tensor(f"x{t}T", [B, D, N], BF16, kind="ExternalInput").ap()
        w[t] = nc.dram_tensor(f"w{t}", [D, DLOC], BF16, kind="ExternalInput").ap()
        bias[t] = nc.dram_tensor(f"b{t}", [DLOC, 1], F32, kind="ExternalInput").ap()
    wo = nc.dram_tensor("wo", [DLOC, D], BF16, kind="ExternalInput").ap()
    outp = nc.dram_tensor("outp", [B * N, D], F32, kind="ExternalOutput").ap()
    zscr = nc.dram_tensor("zscr", [B * NQS, 2 * QS], F16).ap()

    with ExitStack() as ctx:
        tc = ctx.enter_context(tile.TileContext(nc))

        const = ctx.enter_context(tc.tile_pool(name="const", bufs=1))
        xpool = ctx.enter_context(tc.tile_pool(name="xchunks", bufs=16))
        qkpool = ctx.enter_context(tc.tile_pool(name="qk", bufs=4))
        vtpool = ctx.enter_context(tc.tile_pool(name="vt", bufs=2))
        vaugp = ctx.enter_context(tc.tile_pool(name="vaug", bufs=2))
        expp = ctx.enter_context(tc.tile_pool(name="expT", bufs=5))
        zpool = ctx.enter_context(tc.tile_pool(name="zr", bufs=2))
        bcpool = ctx.enter_context(tc.tile_pool(name="bc", bufs=4))
        csbp = ctx.enter_context(tc.tile_pool(name="csb", bufs=4))
        stackp = ctx.enter_context(tc.tile_pool(name="stack", bufs=3))
        hbufp = ctx.enter_context(tc.tile_pool(name="hbuf", bufs=2))
        outsb = ctx.enter_context(tc.tile_pool(name="outsb", bufs=3))

        pp_shared = ctx.enter_context(tc.tile_pool(name="pp_shared", bufs=2, space="PSUM"))
        pp_sc = ctx.enter_context(tc.tile_pool(name="pp_sc", bufs=2, space="PSUM"))
        pp_ctx = ctx.enter_context(tc.tile_pool(name="pp_ctx", bufs=2, space="PSUM"))

        # ---- constants ----
        ident = const.tile([128, 128], BF16, tag="ident")
        make_identity(nc, ident)
        # PE warmup: keep TensorE busy during the initial x-DMA wait so the
        # HAM clock gate reaches full rate before the projections start.
        warm = pp_shared.tile([128, 128], F32, tag="shared", name="warm")
        NWARM = 120
        for i in range(NWARM):
            nc.tensor.matmul(warm, lhsT=ident, rhs=ident,
                             start=(i == 0), stop=(i == NWARM - 1))
        w_sb = {}
        b_sb = {}
        for t in ("q", "k", "v"):
            w_sb[t] = const.tile([128, NDCH, DLOC], BF16, tag=f"w{t}", name=f"w{t}sb")
            nc.sync.dma_start(out=w_sb[t], in_=w[t].rearrange("(c p) m -> p c m", p=128))
            b_sb[t] = const.tile([128, 1], F32, tag=f"b{t}", name=f"b{t}sb")
            nc.sync.dma_start(out=b_sb[t], in_=bias[t])
        wo_sb = const.tile([128, D], BF16, tag="wo")
        nc.sync.dma_start(out=wo_sb, in_=wo)
        ones_c = const.tile([66, QS], F32, tag="onesc")
        nc.vector.memset(ones_c[64:66, :], 1.0)

        # ---- projections ----
        qt_sb = {}   # [b] -> [128, N] bf16  (Q^T, local dims on partitions)
        kt_sb = {}
        vaug = {}    # [b] -> [128, NKT, VW] bf16

        def load_x(b):
            xtiles = {}
            for t in ("q", "k", "v"):
                for c in range(NDCH):
                    xt_ = xpool.tile([128, N], BF16, tag="x", name="xt")
                    nc.sync.dma_start(out=xt_, in_=xT[t][b, ts(c, 128), :])
                    xtiles[(t, c)] = xt_
            return xtiles

        def proj_tgt(t):
            if t == "v":
                return vtpool.tile([128, N], BF16, tag="vt", name="vt_t")
            return qkpool.tile([128, N], BF16, tag="qk", name="qk_t")

        def proj_slice(b, t, xtiles, tgt, s_):
            if True:
                ps = pp_shared.tile([128, QS], F32, tag="shared", name="ps_proj")
                for c in range(NDCH):
                    nc.tensor.matmul(
                        ps,
                        lhsT=w_sb[t][:, c, :],
                        rhs=xtiles[(t, c)][:, ts(s_, QS)],
                        start=(c == 0),
                        stop=(c == NDCH - 1),
                    )
                # bias-add: Scalar engine for b=0 (idle then); DVE for b=1
                # (its projections overlap b=0 attention, where the Scalar
                # engine is busy with exp).
                if b == 0:
                    nc.scalar.activation(
                        tgt[:, ts(s_, QS)], ps,
                        mybir.ActivationFunctionType.Identity,
                        bias=b_sb[t],
                    )
                else:
                    nc.vector.tensor_scalar_add(tgt[:, ts(s_, QS)], ps, b_sb[t])

        def vaug_alloc(b):
            va = vaugp.tile([128, NKT, VW], BF16, tag="vaug", name="va_t")
            nc.vector.memset(va[:, :, 64:65], 0.0)
            nc.vector.memset(va[:, :, 65:66], 1.0)
            nc.vector.memset(va[:, :, 130:131], 1.0)
            vaug[b] = va
            return va

        def vaug_fill(b, tgt, va, tks):
            # transpose V^T -> V (tokens on partitions), pack both heads
            # + denominator columns into the vaug tile.
            for tk in tks:
                pt = pp_shared.tile([128, 128], BF16, tag="shared", name="pt_tr")
                nc.tensor.transpose(pt, tgt[:, ts(tk, 128)], ident)
                # one strided copy: head h -> cols [66h, 66h+64)
                dst = bass.AP(
                    tensor=va.tensor,
                    offset=va.offset + tk * VW,
                    ap=[list(va.ap[0]), [66, 2], [1, 64]],
                )
                src = bass.AP(
                    tensor=pt.tensor,
                    offset=pt.offset,
                    ap=[list(pt.ap[0]), [64, 2], [1, 64]],
                )
                nc.vector.tensor_copy(out=dst, in_=src)

        def proj(b, t, xtiles):
            tgt = proj_tgt(t)
            for s_ in range(NQS):
                proj_slice(b, t, xtiles, tgt, s_)
            if t == "q":
                qt_sb[b] = tgt
            elif t == "k":
                kt_sb[b] = tgt
            else:
                va = vaug_alloc(b)
                vaug_fill(b, tgt, va, range(NKT))

        # ---- attention (wo-phase deferred by one q-slice; static-order
        # anchors keep the scheduler from hoisting wo to the boundary) ----
        pending = []  # [(stack_tile, b, qs), ...]

        def emit_wo_qsub(stack_t, b_, qs_, qsub, anchor=None, use_act=False):
            ob = outsb.tile([128, D], F32, tag="out", name="ob_out")
            for od in range(D // QS):
                pw = pp_shared.tile([128, QS], F32, tag="shared", name="pw_wo")
                mm = nc.tensor.matmul(
                    pw,
                    lhsT=stack_t[:, ts(qsub, 128)],
                    rhs=wo_sb[:, ts(od, QS)],
                    start=True, stop=True,
                )
                if anchor is not None:
                    add_dep_helper(mm.ins, anchor.ins, sync=False,
                                   reason="wo after stack ready")
                # tail flush: the Scalar engine is idle once the last exps
                # are done -- let it do half the PSUM->SBUF copies.
                if use_act and od == 0:
                    nc.scalar.copy(out=ob[:, ts(od, QS)], in_=pw)
                else:
                    nc.vector.tensor_copy(out=ob[:, ts(od, QS)], in_=pw)
                row0 = b_ * N + qs_ * QS + qsub * 128
                nc.sync.dma_start(out=outp[ds(row0, 128), ts(od, QS)],
                                  in_=ob[:, ts(od, QS)])

        def emit_wo(stack_t, b_, qs_, anchors=None, use_act=False):
            for qsub in range(QS // 128):
                a = anchors[qsub] if anchors else None
                emit_wo_qsub(stack_t, b_, qs_, qsub, anchor=a,
                             use_act=use_act)

        def attn_iter(b, qs, mid=None, fillers=None, split_tail=False):
            # psC0 rows: 0-63 ctx0, 64 zero, 65 Z0; psC1: 0-63 ctx1, 64 Z1
            psC0 = pp_ctx.tile([66, QS], F32, tag="ctx", name="ps_ctx0")
            psC1 = pp_ctx.tile([65, QS], F32, tag="ctx", name="ps_ctx1")

            def emit_ctx(kt_, e_):
                nc.tensor.matmul(
                    psC0,
                    lhsT=vaug[b][:, kt_, 0:66],
                    rhs=e_[:, 0:QS],
                    start=(kt_ == 0),
                    stop=(kt_ == NKT - 1),
                )
                nc.tensor.matmul(
                    psC1,
                    lhsT=vaug[b][:, kt_, 66:VW],
                    rhs=e_[:, QS:2 * QS],
                    start=(kt_ == 0),
                    stop=(kt_ == NKT - 1),
                )

            # With mid set (first b0 slice), ALL ctx matmuls run after mid()
            # -- the V projection is emitted between scores and ctx so the
            # attention pipeline starts as soon as Q/K are projected.
            ctxlag = NKT if mid is not None else 0
            e_queue = []
            exp_insts = []
            for kt in range(NKT):
                psS = pp_sc.tile([128, 2 * QS], F32, tag="sc", name="ps_sc")
                nc.tensor.matmul(
                    psS[:, 0:QS],
                    lhsT=kt_sb[b][0:64, ts(kt, 128)],
                    rhs=qt_sb[b][0:64, ts(qs, QS)],
                    start=True, stop=True,
                )
                nc.tensor.matmul(
                    psS[:, QS:2 * QS],
                    lhsT=kt_sb[b][64:128, ts(kt, 128)],
                    rhs=qt_sb[b][64:128, ts(qs, QS)],
                    start=True, stop=True,
                )
                e = expp.tile([128, 2 * QS], BF16, tag="expT", name="e_t")
                exp_inst = nc.scalar.activation(
                    e, psS, mybir.ActivationFunctionType.Exp)
                exp_insts.append(exp_inst)
                e_queue.append((kt, e))
                if len(e_queue) > ctxlag:
                    emit_ctx(*e_queue.pop(0))
                if fillers and kt % 2 == 1:
                    f = fillers.pop(0) if fillers else None
                    if f is not None:
                        f()
            if mid is not None:
                mid()
            while e_queue:
                emit_ctx(*e_queue.pop(0))

            # evacuate full PSUM tiles in ONE copy each (frees psC slots
            # fastest); Z rows are then re-staged from SBUF off the
            # critical PSUM path.
            csb0 = csbp.tile([66, QS], F32, tag="csb", name="csb0")
            csb1 = csbp.tile([65, QS], F32, tag="csb", name="csb1")
            # evacuate the two psC banks on DIFFERENT engines in parallel
            # (Scalar idles at the iteration boundary; DVE takes the other)
            # so both slots free ~1.2us after the last ctx matmul.
            nc.scalar.copy(out=csb0, in_=psC0)
            nc.vector.tensor_copy(out=csb1, in_=psC1)
            # zsb rows (base partition 64): 64 <- Z1, 65 <- Z0.
            zsb = zpool.tile([66, QS], F32, tag="zsb", name="zsb")
            nc.vector.tensor_copy(out=zsb[64:66, :], in_=csb0[64:66, :])
            nc.vector.tensor_copy(out=zsb[64:65, :], in_=csb1[64:65, :])
            zr = zpool.tile([66, QS], F16, tag="zrec", name="zrec")

            # partition-broadcast 1/Z via DRAM roundtrip; for the final
            # q-slice run it in q-halves so the tail wo matmuls can start
            # after only half the chain (deps are column-range precise).
            zrow = zscr[b * NQS + qs, :]
            bc0 = bcpool.tile([64, QS], F16, tag="bc", name="bc0")
            bc1 = bcpool.tile([64, QS], F16, tag="bc", name="bc1")
            stack_t = stackp.tile([128, QS], BF16, tag="stack")
            hb = hbufp.tile([64, QS], BF16, tag="hbuf")
            halves = ((0, QS // 2), (QS // 2, QS)) if split_tail else ((0, QS),)
            for lo, hi in halves:
                w_ = hi - lo
                with nc.allow_low_precision("1/Z in fp16"):
                    nc.vector.reciprocal(out=zr[64:66, lo:hi],
                                         in_=zsb[64:66, lo:hi])
                nc.sync.dma_start(
                    out=bass.AP(tensor=zrow.tensor, offset=zrow.offset + lo,
                                ap=[[QS, 2], [1, w_]]),
                    in_=zr[64:66, lo:hi])
                # zscr row: [0:QS] = 1/Z1 (partition 64), [QS:2QS] = 1/Z0
                for h, bc in ((1, bc0), (0, bc1)):
                    seg = zscr[b * NQS + qs, ds(h * QS + lo, w_)]
                    nc.sync.dma_start(
                        out=bc[:, lo:hi],
                        in_=bass.AP(tensor=seg.tensor, offset=seg.offset,
                                    ap=[[0, 64]] + list(seg.ap)))
                nc.vector.tensor_mul(stack_t[0:64, lo:hi],
                                     csb0[0:64, lo:hi], bc0[:, lo:hi])
                nc.vector.tensor_mul(hb[:, lo:hi], csb1[0:64, lo:hi],
                                     bc1[:, lo:hi])
                # partition shift 0-63 -> 64-127 on DVE
                nc.vector.stream_shuffle(out=stack_t[64:128, lo:hi],
                                         in_=hb[:, lo:hi],
                                         mask=list(range(32)))

            pending.append((stack_t, b, qs))
            if len(pending) > 1:
                anchors = [exp_insts[4], exp_insts[6],
                           exp_insts[8], exp_insts[10]]
                emit_wo(*pending.pop(0), anchors=anchors)

        xt0 = load_x(0)
        proj(0, "q", xt0)
        proj(0, "k", xt0)
        proj(0, "v", xt0)
        xt1 = load_x(1)
        proj(1, "q", xt1)
        proj(1, "k", xt1)
        proj(1, "v", xt1)
        for b in range(B):
            for qs in range(NQS):
                attn_iter(b, qs,
                          split_tail=(b == B - 1 and qs == NQS - 1))

        while pending:
            emit_wo(*pending.pop(0), use_act=True)

    nc.compile()
    return nc


_NC = None


def _get_nc():
    global _NC
    if _NC is None:
        _NC = _build_program()
    return _NC


def _host_prep(query, key, value, Wq, bq, Wk, bk, Wv, bv, Wo, bo):
    bf16 = ml_dtypes.bfloat16
    f32 = np.float32
    q = np.asarray(query, f32)
    k = np.asarray(key, f32)
    v = np.asarray(value, f32)
    Wq = np.asarray(Wq, f32)
    Wk = np.asarray(Wk, f32)
    Wv = np.asarray(Wv, f32)
    Wo = np.asarray(Wo, f32)
    bq = np.asarray(bq, f32)
    bk = np.asarray(bk, f32)
    bv = np.asarray(bv, f32)

    scale = 1.0 / np.sqrt(DH).astype(f32)
    xqT = np.ascontiguousarray(q.transpose(0, 2, 1)).astype(bf16)
    xkT = np.ascontiguousarray(k.transpose(0, 2, 1)).astype(bf16)
    xvT = np.ascontiguousarray(v.transpose(0, 2, 1)).astype(bf16)

    in_maps = []
    for c in range(NCORES):
        sl = slice(c * DLOC, (c + 1) * DLOC)
        in_maps.append({
            "xqT": xqT, "xkT": xkT, "xvT": xvT,
            "wq": np.ascontiguousarray(Wq[:, sl] * scale).astype(bf16),
            "wk": np.ascontiguousarray(Wk[:, sl]).astype(bf16),
            "wv": np.ascontiguousarray(Wv[:, sl]).astype(bf16),
            "bq": np.ascontiguousarray((bq[sl] * scale).reshape(DLOC, 1)),
            "bk": np.ascontiguousarray(bk[sl].reshape(DLOC, 1)),
            "bv": np.ascontiguousarray(bv[sl].reshape(DLOC, 1)),
            "wo": np.ascontiguousarray(Wo[sl, :]).astype(bf16),
        })
    return in_maps


def _run(in_maps, trace=False):
    nc = _get_nc()
    return run_bass_kernel_spmd(nc, in_maps, list(range(NCORES)), trace=trace)


def kernel(query, key, value, Wq, bq, Wk, bk, Wv, bv, Wo, bo):
    in_maps = _host_prep(query, key, value, Wq, bq, Wk, bk, Wv, bv, Wo, bo)
    res = _run(in_maps)
    acc = np.zeros((B * N, D), np.float32)
    for c in range(NCORES):
        acc += np.asarray(res.results[c]["outp"], np.float32)
    acc += np.asarray(bo, np.float32)[None, :]
    return acc.reshape(B, N, D)
